# revision 44
# baseline (speedup 1.0000x reference)
"""Trainium2 Bass kernel for nn_AttentionBlock (GroupNorm + MHA + proj + residual).

Contract: kernel(**inputs) takes the FULL inputs of reference.setup_inputs()
and returns the FULL (8, 512, 32, 32) output. Internally: data-parallel over
the batch dim across 8 NeuronCores (batch == 8, one image per core); weights
are replicated, so no collectives are needed.

Design notes (v7):
  * The ACT-engine exp stream ([128,1024]/slot, ~1.04us x 64) is the spine.
    Per slot the scores matmuls are emitted BEFORE the previous slot's
    AV/filler work: the PE runs its queue in order and exp(s) is gated only
    on scores(s), so scores must never queue behind work that waits on a
    previous exp.  Score PSUM is double-buffered [128,1024]; AV accumulators
    [65,512] rotate 3-deep; one more bank carries the qkv/v/proj fillers.
  * Every big matmul runs fp8e4m3 with MatmulPerfMode.DoubleRow (k-tile
    pairs side by side in the free dim, 16B-aligned strides): qkv/v/proj
    contract 256 channels per 512-column pass, AV contracts 256 j's.  exp is
    written straight to fp8 pair tiles; softmax logits get a free -2.5 bias
    inside the exp so values fit fp8e4m3 (cancels in the normalize).
    q/k stay bf16 (K=64 scores run ~2 cols/cycle anyway).  End-to-end rel
    err vs the f32 reference: ~8.6e-3 (gate 2e-2), verified on host.
  * x is bf16 on the host; all small tensors ship as one packed [128,28]
    f32 tensor; weights are host-quantized fp8 in DoubleRow layout (halves
    the weight DMA).  x chunks lead both HWDGE rings.
  * GroupNorm per 128-channel chunk: sum(x) on the ACT engine (Identity +
    accum_out) in parallel with sum(x^2) on the DVE (tensor_tensor_reduce),
    group-aggregated by one tiny matmul; rsqrt(var+eps) via 2 Newton steps
    on the DVE (group var ~ 1), so the ACT Sqrt table is never loaded and
    the only ACT table load (Exp) happens at boot behind a dummy exp.
  * Junk matmuls (boot + XC-gated + xn-gated) keep the PE from idling
    before the spine: long idle drops the HAM clock and the spine's ~80%
    duty never re-raises it.
  * Softmax normalize: reciprocal straight from the PSUM denominator row,
    gpsimd partition-broadcast, DVE multiply writing fp8 att pair tiles.
    The last window is split into column quarters, all recips first.
  * Tail proj(nh=1): per-u-half PSUM tiles (no false WAR between the u=0
    store and u=1 accumulate), residual x added via an identity matmul on
    the idle PE, proj bias via Identity activation on the idle ACT.
"""

import sys
from contextlib import ExitStack

for _p in ("/opt/trn_rl_repo", "/root/.axon_site/_ro/trn_rl_repo"):
    if _p not in sys.path:
        sys.path.append(_p)

import numpy as np
import ml_dtypes

import concourse.bacc as bacc
import concourse.mybir as mybir
import concourse.tile as tile
from concourse.bass_utils import run_bass_kernel_spmd

F32 = mybir.dt.float32
BF16 = mybir.dt.bfloat16
F8 = mybir.dt.float8e4

B, C, HW = 8, 512, 1024
GROUPS, HEADS, DH = 32, 8, 64
EPS = 1e-5
KC = C // 128
N_CORES = 8
AF = mybir.ActivationFunctionType
OP = mybir.AluOpType
DR = mybir.MatmulPerfMode.DoubleRow
EXP_BIAS = -2.5          # exp(s/8 - 2.5): keeps exp < 240 for fp8e4m3
VSTR = 80                # per-head stride in vt8 free dim (16B-aligned)


def _gn_mats():
    # A8[c, g] = 1/16: applied to per-channel [mean, E[x^2]] (from bn_aggr)
    # it yields group [mean_g, E[x^2]_g].  E8rep replicates the
    # group->channel broadcast at partition offsets 32*kc so each chunk's
    # matmul can use PE tile positions.
    A8 = np.zeros((128, 8), np.float32)
    E8 = np.zeros((8, 128), np.float32)
    for c in range(128):
        A8[c, c // 16] = 1.0 / 16.0
        E8[c // 16, c] = 1.0
    return A8, E8


def _build():
    nc = bacc.Bacc()

    x_h = nc.dram_tensor("xb", [C, HW], BF16, kind="ExternalInput")
    # fp8 DoubleRow weight layouts: [kcp*128+p, u*Ncols+o] with u the k-tile
    # pair index (channel kcp*256 + u*128 + p), host-prepared.
    wqk8_h = nc.dram_tensor("wqk8", [256, 2048], F8, kind="ExternalInput")
    wv8_h = nc.dram_tensor("wv8", [256, 1024], F8, kind="ExternalInput")
    pw8_h = nc.dram_tensor("pw8", [256, 1024], F8, kind="ExternalInput")
    smalls_h = nc.dram_tensor("smalls", [128, 28], F32, kind="ExternalInput")
    y_h = nc.dram_tensor("out", [C, HW], F32, kind="ExternalOutput")

    _, E8_np = _gn_mats()
    E8_h = nc.inline_tensor(E8_np, name="gn_e8")
    eye_h = nc.inline_tensor(np.eye(128, dtype=ml_dtypes.bfloat16), name="eye128")

    with tile.TileContext(nc) as tc, ExitStack() as ctx:
        per = ctx.enter_context(tc.tile_pool(name="per", bufs=1))
        gwork = ctx.enter_context(tc.tile_pool(name="gwork", bufs=2))
        expp = ctx.enter_context(tc.tile_pool(name="expp", bufs=4))
        recp = ctx.enter_context(tc.tile_pool(name="recp", bufs=2))
        outp = ctx.enter_context(tc.tile_pool(name="outp", bufs=4))

        # ---------- persistent tiles ----------
        XC = [per.tile([128, HW], BF16, name=f"XC{i}", tag=f"XC{i}") for i in range(KC)]
        # fp8 pair tiles [128, 2, 1024] (flat [128, 2048]): xn8/att8 by
        # channel-chunk pair, u = chunk parity.
        xn8 = [per.tile([128, 2048], F8, name=f"xn8_{i}", tag=f"xn8_{i}") for i in range(2)]
        att8 = [per.tile([128, 2048], F8, name=f"att8_{i}", tag=f"att8_{i}") for i in range(2)]
        wqk8 = [per.tile([128, 2048], F8, name=f"wqk8_{i}", tag=f"wqk8_{i}") for i in range(2)]
        wv8 = [per.tile([128, 1024], F8, name=f"wv8_{i}", tag=f"wv8_{i}") for i in range(2)]
        pw8 = [per.tile([128, 1024], F8, name=f"pw8_{i}", tag=f"pw8_{i}") for i in range(2)]
        qk = [per.tile([128, HW], BF16, name=f"qk{i}", tag=f"qk{i}") for i in range(8)]
        # fp8 v pair tiles, flat [128 j, 2 j-tiles * 8 heads * VSTR]; head h
        # at [h*VSTR, h*VSTR+64), ones column at h*VSTR+64.
        vt8 = [per.tile([128, 2 * HEADS * VSTR], F8, name=f"vt8_{i}", tag=f"vt8_{i}")
               for i in range(4)]

        smalls = per.tile([128, 28], F32, name="smalls", tag="smalls")
        gnwt = smalls[:, 0:4]
        gnbt = smalls[:, 4:8]
        pbt = smalls[:, 8:12]
        qkbt = smalls[:, 12:20]
        A8t = smalls[:, 20:28]
        E8t = per.tile([8, 128], F32, name="E8t", tag="E8t")
        eps_t = per.tile([8, 1], F32, name="eps", tag="eps")
        expb = per.tile([128, 1], F32, name="expb", tag="expb")
        eyeT = per.tile([128, 128], BF16, name="eyeT", tag="eyeT")
        srt_dummy = per.tile([8, 1], F32, name="srtd", tag="srtd")
        scr = per.tile([128, 512], BF16, name="scr", tag="scr")

        # ---------- input DMAs (x first on both HWDGE rings) ----------
        nc.sync.dma_start(out=XC[0], in_=x_h[0:128, :])
        nc.sync.dma_start(out=XC[2], in_=x_h[256:384, :])
        nc.sync.dma_start(out=smalls, in_=smalls_h[:, :])
        nc.scalar.dma_start(out=XC[1], in_=x_h[128:256, :])
        nc.scalar.dma_start(out=XC[3], in_=x_h[384:512, :])
        nc.scalar.dma_start(out=E8t, in_=E8_h[:, :])
        for kcp in range(2):
            eng = nc.sync if kcp == 0 else nc.scalar
            eng.dma_start(out=wqk8[kcp], in_=wqk8_h[kcp * 128:(kcp + 1) * 128, :])
            eng.dma_start(out=wv8[kcp], in_=wv8_h[kcp * 128:(kcp + 1) * 128, :])
            eng.dma_start(out=pw8[kcp], in_=pw8_h[kcp * 128:(kcp + 1) * 128, :])
        nc.sync.dma_start(out=eyeT, in_=eye_h[:, :])

        # junk-matmul source on the otherwise-idle-at-boot vector engine
        nc.vector.memset(scr, 0.001)
        for jp in range(4):
            for u in range(2):
                vv = vt8[jp][:, u * HEADS * VSTR:(u + 1) * HEADS * VSTR]
                v3 = vv.rearrange("p (h e) -> p h e", e=VSTR)
                nc.gpsimd.memset(v3[:, :, DH:DH + 1], 1.0)
        nc.gpsimd.memset(eps_t, EPS)
        nc.gpsimd.memset(expb, EXP_BIAS)
        # dummy exp: pulls the (only) ACT table load to boot, where ACT idles
        nc.scalar.activation(out=srt_dummy, in_=eps_t, func=AF.Exp,
                             bias=eps_t[:], scale=1.0)

        def v3d(t2048):
            return t2048[:].rearrange("p (u n) -> p u n", u=2)

        # ---------- groupnorm (per-kc) + HAM warmup ----------
        with tc.tile_pool(name="ps_gn", bufs=2, space="PSUM") as ps_gn, \
             tc.tile_pool(name="ps_wm", bufs=1, space="PSUM") as ps_wm:
            wmt = ps_wm.tile([128, 512], F32, name="wmt", tag="wmt")

            def junk(n):
                for _ in range(n):
                    nc.tensor.matmul(wmt[:], scr[:, 0:128], scr[:],
                                     start=True, stop=True)

            junk(8)
            # XC-gated junk: keeps the PE busy while x lands and GN runs.
            for kc in range(KC):
                for _ in range(4 if kc < 3 else 6):
                    nc.tensor.matmul(wmt[:], XC[kc][:, 0:128], scr[:],
                                     start=True, stop=True)

            mes = []
            for kc in range(KC):
                me = gwork.tile([128, 2], F32, name="me", tag=f"me{kc}")
                stats = gwork.tile([128, 2, 6], F32, name="st", tag="st")
                xv = XC[kc][:].rearrange("p (s f) -> p s f", f=512)
                for shalf in range(2):
                    nc.vector.bn_stats(out=stats[:, shalf, :],
                                       in_=xv[:, shalf, :])
                mv = gwork.tile([128, 2], F32, name="mv", tag="mv")
                nc.vector.bn_aggr(out=mv, in_=stats)
                nc.vector.tensor_mul(out=me[:, 1:2], in0=mv[:, 0:1],
                                     in1=mv[:, 0:1])
                nc.vector.tensor_add(out=me[:, 1:2], in0=me[:, 1:2],
                                     in1=mv[:, 1:2])
                nc.vector.tensor_copy(out=me[:, 0:1], in_=mv[:, 0:1])
                mes.append(me)
                for _ in range(3):
                    nc.tensor.matmul(wmt[:], XC[kc][:, 256:384], scr[:],
                                     start=True, stop=True)
            # one batched chain over all 32 groups: chunk kc's group stats
            # live at COLUMNS 2kc..2kc+2 of an [8, 8] tile (all matmuls at
            # partition base 0); even columns = mean, odd = E[x^2].
            gstat = ps_gn.tile([8, 8], F32, name="gstat", tag="gstat")
            for kc in range(KC):
                nc.tensor.matmul(gstat[:, 2 * kc:2 * kc + 2], A8t,
                                 mes[kc][:], start=True, stop=True,
                                 skip_group_check=True)
            gs = gwork.tile([8, 8], F32, name="gs", tag="gs")
            nc.vector.tensor_copy(out=gs, in_=gstat)
            # chain-gated junk: a 1-element DVE write into scr creates a RAW
            # dep, so these matmuls execute HERE (not hoisted to boot),
            # covering the PE through the groupnorm scalar chain.
            nc.vector.tensor_copy(out=scr[0:1, 0:1], in_=gs[0:1, 0:1])
            junk(8)

            def ev(t):
                return t[:].rearrange("p (k two) -> p k two", two=2)[:, :, 0:1]

            def od(t):
                return t[:].rearrange("p (k two) -> p k two", two=2)[:, :, 1:2]

            def v41(t):
                return t[:].rearrange("p (k one) -> p k one", one=1)

            var = gwork.tile([8, 4], F32, name="var", tag="var")
            nc.vector.tensor_mul(out=v41(var), in0=ev(gs), in1=ev(gs))
            nc.vector.tensor_sub(out=v41(var), in0=od(gs), in1=v41(var))
            # rsqrt(var+eps): 2 Newton steps from y0=1 (GN group var ~ 1)
            w = gwork.tile([8, 4], F32, name="wv", tag="wv")
            nc.vector.tensor_scalar(out=w, in0=var, scalar1=EPS,
                                    scalar2=None, op0=OP.add)
            # Newton's y overwrites the E[x^2] columns of gs in place, so gs
            # itself becomes the [mean, rsqrt] matmul rhs (no extra copies).
            y = od(gs)
            tt = gwork.tile([8, 4], F32, name="tt", tag="tt")
            nc.vector.tensor_scalar(out=y, in0=v41(w), scalar1=-0.5,
                                    scalar2=1.5, op0=OP.mult, op1=OP.add)
            for _ in range(2):
                nc.vector.tensor_mul(out=v41(tt), in0=y, in1=y)
                nc.vector.tensor_mul(out=v41(tt), in0=v41(tt), in1=v41(w))
                nc.vector.tensor_scalar(out=tt, in0=tt, scalar1=-0.5,
                                        scalar2=1.5, op0=OP.mult,
                                        op1=OP.add)
                nc.vector.tensor_mul(out=y, in0=y, in1=v41(tt))
            cb = ps_gn.tile([128, 8], F32, name="cb", tag="cb")
            for kc in range(KC):
                nc.tensor.matmul(cb[:, 2 * kc:2 * kc + 2],
                                 E8t[:],
                                 gs[:, 2 * kc:2 * kc + 2],
                                 start=True, stop=True,
                                 skip_group_check=True)
            nc.vector.tensor_copy(out=scr[0:1, 0:1], in_=gs[0:1, 0:1])
            junk(6)
            for kc in range(KC):
                sc = gwork.tile([128, 1], F32, name=f"sc{kc}", tag=f"sc{kc}")
                sh = gwork.tile([128, 1], F32, name=f"sh{kc}", tag=f"sh{kc}")
                nc.vector.tensor_mul(out=sc, in0=cb[:, 2 * kc + 1:2 * kc + 2],
                                     in1=gnwt[:, kc:kc + 1])
                nc.vector.tensor_mul(out=sh, in0=cb[:, 2 * kc:2 * kc + 1],
                                     in1=sc)
                nc.vector.tensor_sub(out=sh, in0=gnbt[:, kc:kc + 1], in1=sh)
                if kc % 2 == 0:
                    nc.scalar.activation(
                        out=xn8[kc // 2][:, (kc % 2) * 1024:(kc % 2 + 1) * 1024],
                        in_=XC[kc][:], func=AF.Identity,
                        bias=sh[:], scale=sc[:])
                else:
                    nc.vector.tensor_scalar(
                        out=xn8[kc // 2][:, (kc % 2) * 1024:(kc % 2 + 1) * 1024],
                        in0=XC[kc][:], scalar1=sc[:], scalar2=sh[:],
                        op0=OP.mult, op1=OP.add)

        # ---------- qkv / attention / proj ----------
        with tc.tile_pool(name="ps_q", bufs=1, space="PSUM") as ps_q:
            with tc.tile_pool(name="ps_wm2", bufs=1, space="PSUM") as ps_wm2:
                wmt2 = ps_wm2.tile([128, 512], F32, name="wmt2", tag="wmt2")
                for _ in range(6):
                    nc.tensor.matmul(
                        wmt2[:],
                        xn8[0][:].rearrange("p (u n) -> p u n", u=2)[:, :, 0:128],
                        xn8[0][:].rearrange("p (u n) -> p u n", u=2)[:, :, 0:512],
                        start=True, stop=True,
                        perf_mode=DR, skip_group_check=True)
            def emit_qk_half(oc, nh, pool, tag, act=False):
                pq = pool.tile([128, 512], F32, name="pq", tag=tag)
                for kcp in range(2):
                    nc.tensor.matmul(
                        pq[:],
                        v3d(wqk8[kcp])[:, :, oc * 128:(oc + 1) * 128],
                        v3d(xn8[kcp])[:, :, nh * 512:(nh + 1) * 512],
                        start=(kcp == 0), stop=(kcp == 1),
                        perf_mode=DR, skip_group_check=True)
                    yield
                if act:
                    # head phase: bias-add on the still-idle ACT engine
                    nc.scalar.activation(
                        out=qk[oc][:, nh * 512:(nh + 1) * 512], in_=pq[:],
                        func=AF.Identity, bias=qkbt[:, oc:oc + 1], scale=1.0)
                else:
                    nc.vector.tensor_scalar(
                        out=qk[oc][:, nh * 512:(nh + 1) * 512],
                        in0=pq[:], scalar1=qkbt[:, oc:oc + 1],
                        scalar2=None, op0=OP.add)
                yield

            def emit_vt(jc, pool, tag, act=False):
                pv = pool.tile([128, 512], F32, name="pv", tag=tag)
                for kcp in range(2):
                    nc.tensor.matmul(
                        pv[:],
                        v3d(xn8[kcp])[:, :, jc * 128:(jc + 1) * 128],
                        v3d(wv8[kcp]),
                        start=(kcp == 0), stop=(kcp == 1),
                        perf_mode=DR, skip_group_check=True)
                    yield
                u = jc % 2
                vv = vt8[jc // 2][:, u * HEADS * VSTR:(u + 1) * HEADS * VSTR]
                v3 = vv.rearrange("p (h e) -> p h e", e=VSTR)
                if act:
                    nc.scalar.activation(
                        out=v3[:, :, 0:DH],
                        in_=pv[:].rearrange("p (h d) -> p h d", h=HEADS),
                        func=AF.Identity, bias=0.0, scale=1.0)
                else:
                    nc.vector.tensor_copy(
                        out=v3[:, :, 0:DH],
                        in_=pv[:].rearrange("p (h d) -> p h d", h=HEADS))
                yield

            def emit_proj_half(oc, nh, pool, tag):
                pp = pool.tile([128, 512], F32, name="pp", tag=tag)
                for prp in range(2):
                    nc.tensor.matmul(
                        pp[:],
                        v3d(pw8[prp])[:, :, oc * 128:(oc + 1) * 128],
                        v3d(att8[prp])[:, :, nh * 512:(nh + 1) * 512],
                        start=(prp == 0), stop=False,
                        perf_mode=DR, skip_group_check=True)
                    yield
                nc.tensor.matmul(
                    pp[:], eyeT[:], XC[oc][:, nh * 512:(nh + 1) * 512],
                    start=False, stop=True, skip_group_check=True)
                ot = outp.tile([128, 512], F32, name="ot", tag="ot")
                nc.vector.tensor_scalar(out=ot[:], in0=pp[:],
                                        scalar1=pbt[:, oc:oc + 1],
                                        scalar2=None, op0=OP.add)
                # mid-spine stores go via gpsimd SWDGE: a ~0.6us HWDGE
                # descriptor-gen on the scalar sequencer would delay exp
                # dispatch (sync is the semaphore hub, also avoided)
                nc.gpsimd.dma_start(
                    out=y_h[oc * 128:(oc + 1) * 128, nh * 512:(nh + 1) * 512],
                    in_=ot[:])
                yield

            def emit_filler(gen, n):
                k = 0
                for _ in range(n):
                    try:
                        next(gen)
                        k += 1
                    except StopIteration:
                        break
                return k


            # upfront: q0-nh0, k0 both halves, v0..v5
            with tc.tile_pool(name="ps_pre", bufs=4, space="PSUM") as ps_pre:
                for oc, nh, act in ((0, 0, False), (4, 0, True), (4, 1, False)):
                    emit_filler(emit_qk_half(oc, nh, ps_pre, "ppre", act=act), 99)
                for jc in range(6):
                    emit_filler(emit_vt(jc, ps_pre, "ppre", act=(jc < 2)), 99)

            def qk_stream():
                yield from emit_qk_half(0, 1, ps_q, "pq")
                for jc in (6, 7):
                    yield from emit_vt(jc, ps_q, "pq")
                for oc, nh in ((5, 0), (5, 1), (1, 0),
                               (1, 1), (6, 0), (6, 1), (2, 0),
                               (2, 1), (7, 0), (7, 1), (3, 0), (3, 1)):
                    yield from emit_qk_half(oc, nh, ps_q, "pq")

            def proj0_stream():
                for oc in range(KC):
                    yield from emit_proj_half(oc, 0, ps_q, "pq")

            fill_qk = qk_stream()
            fill_proj = proj0_stream()

            ps_av_cm = tc.tile_pool(name="ps_av", bufs=3, space="PSUM")
            ps_av = ps_av_cm.__enter__()
            ps_s_cm = tc.tile_pool(name="ps_s", bufs=2, space="PSUM")
            ps_s = ps_s_cm.__enter__()
            if True:

                win = {}       # w -> [pav0, pav1]
                exq = {}       # global pair index -> fp8 exp pair tile

                def new_window(w):
                    win[w] = [ps_av.tile([DH + 1, 512], F32, name=f"pav{t}",
                                         tag="pav") for t in range(2)]

                def emit_av(p, ts=(0, 1)):
                    w, jcp = p // 4, p % 4
                    pr = w // 2
                    ex2 = v3d(exq[p])
                    v2 = vt8[jcp][:].rearrange("p (u f) -> p u f", u=2)
                    for t in ts:
                        h = 2 * pr + t
                        nc.tensor.matmul(
                            win[w][t][:],
                            v2[:, :, h * VSTR:h * VSTR + DH + 1],
                            ex2[:, :, t * 512:(t + 1) * 512],
                            start=(jcp == 0), stop=(jcp == 3),
                            perf_mode=DR,
                            skip_group_check=True)

                def emit_norm(w):
                    pav = win.pop(w)
                    pr, hf = w // 2, w % 2
                    base = (pr % 2) * 1024 + hf * 512
                    for t in range(2):
                        dn = recp.tile([1, 512], F32, name=f"den{t}",
                                       tag=f"den{t}")
                        nc.vector.tensor_copy(out=dn[:],
                                              in_=pav[t][DH:DH + 1, :])
                        rc = recp.tile([1, 512], F32, name=f"rec{t}",
                                       tag=f"rec{t}")
                        nc.vector.reciprocal_approx_fast(
                            out=rc[:], in_=dn[:])
                        rb = recp.tile([DH, 512], F32, name=f"rb{t}",
                                       tag=f"rb{t}")
                        nc.gpsimd.partition_broadcast(out_ap=rb[:], in_ap=rc[:])
                        nc.vector.tensor_mul(
                            out=att8[pr // 2][64 * t:64 * t + DH,
                                              base:base + 512],
                            in0=pav[t][0:DH, :],
                            in1=rb[:])

                pend = []
                for s in range(64):
                    w, jc = s // 8, s % 8
                    pr, hf = w // 2, w % 2
                    if jc == 0:
                        new_window(w)
                    qt, kt = qk[pr], qk[4 + pr]
                    pss = ps_s.tile([128, HW], F32, name="pss", tag="pss")
                    for t in range(2):
                        nc.tensor.matmul(
                            pss[:, t * 512:(t + 1) * 512],
                            kt[64 * t:64 * t + DH, jc * 128:(jc + 1) * 128],
                            qt[64 * t:64 * t + DH, hf * 512:(hf + 1) * 512],
                            start=True, stop=True)
                    for f in pend:
                        f()
                    pend = []
                    p = s // 2
                    if s % 2 == 0:
                        exq[p] = expp.tile([128, 2048], F8, name="expT",
                                           tag="expT")
                    ub = (s % 2) * 1024
                    if s == 63:
                        for t in range(2):
                            nc.scalar.activation(
                                out=exq[p][:, ub + t * 512:ub + (t + 1) * 512],
                                in_=pss[:, t * 512:(t + 1) * 512],
                                func=AF.Exp, scale=float(DH) ** -0.5,
                                bias=expb[:])
                    else:
                        nc.scalar.activation(out=exq[p][:, ub:ub + 1024],
                                             in_=pss[:],
                                             func=AF.Exp,
                                             scale=float(DH) ** -0.5,
                                             bias=expb[:])
                    if s >= 2 and s % 2 == 0:
                        pe = (s - 2) // 2
                        pend.append(lambda pe=pe: emit_av(pe, ts=(0,)))
                    elif s >= 3:
                        pe = (s - 3) // 2
                        pend.append(lambda pe=pe: (emit_av(pe, ts=(1,)),
                                                   exq.pop(pe)))
                        if pe % 4 == 3 and pe // 4 < 7:
                            pend.append(lambda ww=pe // 4: emit_norm(ww))
                    if s == 63:
                        pass
                    elif w == 7 and jc >= 1:
                        pend.append(lambda: emit_filler(fill_proj, 2))
                    else:
                        n = 2 if s < 6 else (1 if s % 2 == 0 else 2)
                        pend.append(lambda n=n: emit_filler(fill_qk, n))
                for f in pend:
                    f()

                # tail: last AV pair + staged final normalize: all recips
                # (straight from the PSUM denominator rows) first, then
                # broadcast+multiply per quarter, u-major so the u=0 half of
                # att[3] is released first for the split proj matmuls.
                emit_av(31)
                exq.pop(31)
                pav7 = win.pop(7)
                n_rc = []
                for u in range(2):
                    for t in range(2):
                        dn = recp.tile([1, 512], F32, name=f"den{t}", tag=f"den{t}")
                        nc.scalar.activation(
                            out=dn[:, u * 256:(u + 1) * 256],
                            in_=pav7[t][DH:DH + 1, u * 256:(u + 1) * 256],
                            func=AF.Identity, bias=0.0, scale=1.0)
                        rc = recp.tile([1, 512], F32, name=f"rec{t}", tag=f"rec{t}")
                        nc.vector.reciprocal_approx_fast(
                            out=rc[:, u * 256:(u + 1) * 256],
                            in_=dn[:, u * 256:(u + 1) * 256])
                        n_rc.append(rc)
                for i, (u, t) in enumerate(((0, 0), (0, 1), (1, 0), (1, 1))):
                    rb = recp.tile([DH, 512], F32, name=f"rb{t}", tag=f"rb{t}")
                    nc.gpsimd.partition_broadcast(
                        out_ap=rb[:, u * 256:(u + 1) * 256],
                        in_ap=n_rc[i][:, u * 256:(u + 1) * 256])
                    nc.vector.tensor_mul(
                        out=att8[1][64 * t:64 * t + DH,
                                    1536 + u * 256:1536 + (u + 1) * 256],
                        in0=pav7[t][0:DH, u * 256:(u + 1) * 256],
                        in1=rb[:, u * 256:(u + 1) * 256])
                emit_filler(fill_qk, 1000)
                emit_filler(fill_proj, 1000)

            ps_s_cm.__exit__(None, None, None)
            # proj nh=1 in the 4 banks just freed by ps_s (ps_av still open,
            # so the allocator cannot overlap the pav banks -- their release
            # waits on the final normalize reads and would stall these
            # matmuls behind it).
            with tc.tile_pool(name="ps_p2", bufs=4, space="PSUM") as ps_p2:
                pps = [ps_p2.tile([128, 512], F32, name="pp", tag="pp2")
                       for _ in range(KC)]
                for oc in range(KC):
                    nc.tensor.matmul(
                        pps[oc][:],
                        v3d(pw8[0])[:, :, oc * 128:(oc + 1) * 128],
                        v3d(att8[0])[:, :, 512:1024],
                        start=True, stop=False,
                        perf_mode=DR, skip_group_check=True)
                ots = [outp.tile([128, 512], F32, name="ot", tag="ot")
                       for _ in range(KC)]
                for u in range(2):
                    for oc in range(KC):
                        nc.tensor.matmul(
                            pps[oc][:, u * 256:(u + 1) * 256],
                            v3d(pw8[1])[:, :, oc * 128:(oc + 1) * 128],
                            v3d(att8[1])[:, :, 512 + u * 256:512 + (u + 1) * 256],
                            start=False, stop=False,
                            perf_mode=DR, skip_group_check=True)
                    for oc in range(KC):
                        nc.tensor.matmul(
                            pps[oc][:, u * 256:(u + 1) * 256],
                            eyeT[:],
                            XC[oc][:, 512 + u * 256:512 + (u + 1) * 256],
                            start=False, stop=True,
                            skip_group_check=True)
                for u in range(2):
                    for oc in range(KC):
                        if u == 0:
                            nc.vector.tensor_scalar(
                                out=ots[oc][:, 0:256],
                                in0=pps[oc][:, 0:256],
                                scalar1=pbt[:, oc:oc + 1],
                                scalar2=None, op0=OP.add)
                        else:
                            nc.scalar.activation(
                                out=ots[oc][:, 256:512],
                                in_=pps[oc][:, 256:512],
                                func=AF.Identity, bias=pbt[:, oc:oc + 1],
                                scale=1.0)
                        eng = nc.sync if (oc + u) % 2 == 0 else nc.scalar
                        eng.dma_start(
                            out=y_h[oc * 128:(oc + 1) * 128,
                                    512 + u * 256:512 + (u + 1) * 256],
                            in_=ots[oc][:, u * 256:(u + 1) * 256])
            ps_av_cm.__exit__(None, None, None)
    nc.compile()
    return nc


_NC = None


def _get_nc():
    global _NC
    if _NC is None:
        _NC = _build()
    return _NC


def _dr_pack(wT):
    """[512 c, N] -> DoubleRow fp8 layout [2*128, 2*N]:
    out[kcp*128+p, u*N+o] = wT[kcp*256+u*128+p, o]."""
    n = wT.shape[1]
    w4 = wT.reshape(2, 2, 128, n).transpose(0, 2, 1, 3).reshape(256, 2 * n)
    return np.ascontiguousarray(w4).astype(ml_dtypes.float8_e4m3)


def _run(inputs, **kwargs):
    nc = _get_nc()
    x = np.asarray(inputs["x"], dtype=np.float32)
    qkv_w = np.asarray(inputs["qkv_w"], np.float32)
    proj_w = np.asarray(inputs["proj_w"], np.float32)
    qkv_b = np.asarray(inputs["qkv_b"], np.float32)
    pb_eff = (np.asarray(inputs["proj_b"], np.float32)
              + proj_w @ qkv_b[1024:1536])
    A8_np, _ = _gn_mats()
    smalls = np.empty((128, 28), np.float32)
    smalls[:, 0:4] = np.asarray(inputs["gn_w"], np.float32).reshape(KC, 128).T
    smalls[:, 4:8] = np.asarray(inputs["gn_b"], np.float32).reshape(KC, 128).T
    smalls[:, 8:12] = pb_eff.reshape(KC, 128).T
    smalls[:, 12:20] = qkv_b[0:1024].reshape(8, 128).T
    smalls[:, 20:28] = A8_np
    shared = {
        "wqk8": _dr_pack(np.ascontiguousarray(qkv_w[0:1024].T)),
        "wv8": _dr_pack(np.ascontiguousarray(qkv_w[1024:1536].T)),
        "pw8": _dr_pack(np.ascontiguousarray(proj_w.T)),
        "smalls": smalls,
    }
    xb = x.reshape(B, C, HW).astype(ml_dtypes.bfloat16)
    in_maps = [dict(shared, xb=np.ascontiguousarray(xb[m])) for m in range(B)]
    res = run_bass_kernel_spmd(nc, in_maps, core_ids=list(range(N_CORES)), **kwargs)
    out = np.stack([res.results[m]["out"] for m in range(B)])
    return out.reshape(B, C, 32, 32).astype(np.float32), res


def kernel(**inputs):
    out, _ = _run(inputs)
    return out


# revision 46
# speedup vs baseline: 1.0179x; 1.0179x over previous
"""Trainium2 Bass kernel for nn_AttentionBlock (GroupNorm + MHA + proj + residual).

Contract: kernel(**inputs) takes the FULL inputs of reference.setup_inputs()
and returns the FULL (8, 512, 32, 32) output. Internally: data-parallel over
the batch dim across 8 NeuronCores (batch == 8, one image per core); weights
are replicated, so no collectives are needed.

Design notes (v7):
  * The ACT-engine exp stream ([128,1024]/slot, ~1.04us x 64) is the spine.
    Per slot the scores matmuls are emitted BEFORE the previous slot's
    AV/filler work: the PE runs its queue in order and exp(s) is gated only
    on scores(s), so scores must never queue behind work that waits on a
    previous exp.  Score PSUM is double-buffered [128,1024]; AV accumulators
    [65,512] rotate 3-deep; one more bank carries the qkv/v/proj fillers.
  * Every big matmul runs fp8e4m3 with MatmulPerfMode.DoubleRow (k-tile
    pairs side by side in the free dim, 16B-aligned strides): qkv/v/proj
    contract 256 channels per 512-column pass, AV contracts 256 j's.  exp is
    written straight to fp8 pair tiles; softmax logits get a free -2.5 bias
    inside the exp so values fit fp8e4m3 (cancels in the normalize).
    q/k stay bf16 (K=64 scores run ~2 cols/cycle anyway).  End-to-end rel
    err vs the f32 reference: ~8.6e-3 (gate 2e-2), verified on host.
  * x is bf16 on the host; all small tensors ship as one packed [128,28]
    f32 tensor; weights are host-quantized fp8 in DoubleRow layout (halves
    the weight DMA).  x chunks lead both HWDGE rings.
  * GroupNorm per 128-channel chunk: sum(x) on the ACT engine (Identity +
    accum_out) in parallel with sum(x^2) on the DVE (tensor_tensor_reduce),
    group-aggregated by one tiny matmul; rsqrt(var+eps) via 2 Newton steps
    on the DVE (group var ~ 1), so the ACT Sqrt table is never loaded and
    the only ACT table load (Exp) happens at boot behind a dummy exp.
  * Junk matmuls (boot + XC-gated + xn-gated) keep the PE from idling
    before the spine: long idle drops the HAM clock and the spine's ~80%
    duty never re-raises it.
  * Softmax normalize: reciprocal straight from the PSUM denominator row,
    gpsimd partition-broadcast, DVE multiply writing fp8 att pair tiles.
    The last window is split into column quarters, all recips first.
  * Windows run hf-major ((pr,hf) = (w%4, w//4)), so all nh=0 attention
    finishes by slot ~33 and proj(nh=0) spreads over the late spine's
    otherwise-empty filler slots (keeps PE duty up; the clock decays on
    real HW when duty drops, which then stretches the exp stream).
  * Tail proj(nh=1): proj PSUM forced into the freed score banks (never
    the AV banks, whose release waits on the final normalize), residual x
    added via an identity matmul on the idle PE, proj bias via Identity
    activation on the idle ACT.
"""

import sys
from contextlib import ExitStack

for _p in ("/opt/trn_rl_repo", "/root/.axon_site/_ro/trn_rl_repo"):
    if _p not in sys.path:
        sys.path.append(_p)

import numpy as np
import ml_dtypes

import concourse.bacc as bacc
import concourse.mybir as mybir
import concourse.tile as tile
from concourse.bass_utils import run_bass_kernel_spmd

F32 = mybir.dt.float32
BF16 = mybir.dt.bfloat16
F8 = mybir.dt.float8e4

B, C, HW = 8, 512, 1024
GROUPS, HEADS, DH = 32, 8, 64
EPS = 1e-5
KC = C // 128
N_CORES = 8
AF = mybir.ActivationFunctionType
OP = mybir.AluOpType
DR = mybir.MatmulPerfMode.DoubleRow
EXP_BIAS = -2.5          # exp(s/8 - 2.5): keeps exp < 240 for fp8e4m3
VSTR = 80                # per-head stride in vt8 free dim (16B-aligned)


def _gn_mats():
    # A8[c, g] = 1/16: applied to per-channel [mean, E[x^2]] (from bn_aggr)
    # it yields group [mean_g, E[x^2]_g].  E8rep replicates the
    # group->channel broadcast at partition offsets 32*kc so each chunk's
    # matmul can use PE tile positions.
    A8 = np.zeros((128, 8), np.float32)
    E8 = np.zeros((8, 128), np.float32)
    for c in range(128):
        A8[c, c // 16] = 1.0 / 16.0
        E8[c // 16, c] = 1.0
    return A8, E8


def _build():
    nc = bacc.Bacc()

    x_h = nc.dram_tensor("xb", [C, HW], BF16, kind="ExternalInput")
    # fp8 DoubleRow weight layouts: [kcp*128+p, u*Ncols+o] with u the k-tile
    # pair index (channel kcp*256 + u*128 + p), host-prepared.
    wqk8_h = nc.dram_tensor("wqk8", [256, 2048], F8, kind="ExternalInput")
    wv8_h = nc.dram_tensor("wv8", [256, 1024], F8, kind="ExternalInput")
    pw8_h = nc.dram_tensor("pw8", [256, 1024], F8, kind="ExternalInput")
    smalls_h = nc.dram_tensor("smalls", [128, 28], F32, kind="ExternalInput")
    y_h = nc.dram_tensor("out", [C, HW], F32, kind="ExternalOutput")

    _, E8_np = _gn_mats()
    E8_h = nc.inline_tensor(E8_np, name="gn_e8")
    eye_h = nc.inline_tensor(np.eye(128, dtype=ml_dtypes.bfloat16), name="eye128")

    with tile.TileContext(nc) as tc, ExitStack() as ctx:
        per = ctx.enter_context(tc.tile_pool(name="per", bufs=1))
        gwork = ctx.enter_context(tc.tile_pool(name="gwork", bufs=2))
        expp = ctx.enter_context(tc.tile_pool(name="expp", bufs=4))
        recp = ctx.enter_context(tc.tile_pool(name="recp", bufs=2))
        outp = ctx.enter_context(tc.tile_pool(name="outp", bufs=4))

        # ---------- persistent tiles ----------
        XC = [per.tile([128, HW], BF16, name=f"XC{i}", tag=f"XC{i}") for i in range(KC)]
        # fp8 pair tiles [128, 2, 1024] (flat [128, 2048]): xn8/att8 by
        # channel-chunk pair, u = chunk parity.
        xn8 = [per.tile([128, 2048], F8, name=f"xn8_{i}", tag=f"xn8_{i}") for i in range(2)]
        att8 = [per.tile([128, 2048], F8, name=f"att8_{i}", tag=f"att8_{i}") for i in range(2)]
        wqk8 = [per.tile([128, 2048], F8, name=f"wqk8_{i}", tag=f"wqk8_{i}") for i in range(2)]
        wv8 = [per.tile([128, 1024], F8, name=f"wv8_{i}", tag=f"wv8_{i}") for i in range(2)]
        pw8 = [per.tile([128, 1024], F8, name=f"pw8_{i}", tag=f"pw8_{i}") for i in range(2)]
        qk = [per.tile([128, HW], BF16, name=f"qk{i}", tag=f"qk{i}") for i in range(8)]
        # fp8 v pair tiles, flat [128 j, 2 j-tiles * 8 heads * VSTR]; head h
        # at [h*VSTR, h*VSTR+64), ones column at h*VSTR+64.
        vt8 = [per.tile([128, 2 * HEADS * VSTR], F8, name=f"vt8_{i}", tag=f"vt8_{i}")
               for i in range(4)]

        smalls = per.tile([128, 28], F32, name="smalls", tag="smalls")
        gnwt = smalls[:, 0:4]
        gnbt = smalls[:, 4:8]
        pbt = smalls[:, 8:12]
        qkbt = smalls[:, 12:20]
        A8t = smalls[:, 20:28]
        E8t = per.tile([8, 128], F32, name="E8t", tag="E8t")
        eps_t = per.tile([8, 1], F32, name="eps", tag="eps")
        expb = per.tile([128, 1], F32, name="expb", tag="expb")
        eyeT = per.tile([128, 128], BF16, name="eyeT", tag="eyeT")
        srt_dummy = per.tile([8, 1], F32, name="srtd", tag="srtd")
        scr = per.tile([128, 512], BF16, name="scr", tag="scr")

        # ---------- input DMAs (x first on both HWDGE rings) ----------
        nc.sync.dma_start(out=XC[0], in_=x_h[0:128, :])
        nc.sync.dma_start(out=XC[2], in_=x_h[256:384, :])
        nc.sync.dma_start(out=smalls, in_=smalls_h[:, :])
        nc.scalar.dma_start(out=XC[1], in_=x_h[128:256, :])
        nc.scalar.dma_start(out=XC[3], in_=x_h[384:512, :])
        nc.scalar.dma_start(out=E8t, in_=E8_h[:, :])
        for kcp in range(2):
            eng = nc.sync if kcp == 0 else nc.scalar
            eng.dma_start(out=wqk8[kcp], in_=wqk8_h[kcp * 128:(kcp + 1) * 128, :])
            eng.dma_start(out=wv8[kcp], in_=wv8_h[kcp * 128:(kcp + 1) * 128, :])
            eng.dma_start(out=pw8[kcp], in_=pw8_h[kcp * 128:(kcp + 1) * 128, :])
        nc.sync.dma_start(out=eyeT, in_=eye_h[:, :])

        # junk-matmul source on the otherwise-idle-at-boot vector engine
        nc.vector.memset(scr, 0.001)
        for jp in range(4):
            for u in range(2):
                vv = vt8[jp][:, u * HEADS * VSTR:(u + 1) * HEADS * VSTR]
                v3 = vv.rearrange("p (h e) -> p h e", e=VSTR)
                nc.gpsimd.memset(v3[:, :, DH:DH + 1], 1.0)
        nc.gpsimd.memset(eps_t, EPS)
        nc.gpsimd.memset(expb, EXP_BIAS)
        # dummy exp: pulls the (only) ACT table load to boot, where ACT idles
        nc.scalar.activation(out=srt_dummy, in_=eps_t, func=AF.Exp,
                             bias=eps_t[:], scale=1.0)

        def v3d(t2048):
            return t2048[:].rearrange("p (u n) -> p u n", u=2)

        # ---------- groupnorm (per-kc) + HAM warmup ----------
        with tc.tile_pool(name="ps_gn", bufs=2, space="PSUM") as ps_gn, \
             tc.tile_pool(name="ps_wm", bufs=1, space="PSUM") as ps_wm:
            wmt = ps_wm.tile([128, 512], F32, name="wmt", tag="wmt")

            def junk(n):
                for _ in range(n):
                    nc.tensor.matmul(wmt[:], scr[:, 0:128], scr[:],
                                     start=True, stop=True)

            junk(8)
            # XC-gated junk: keeps the PE busy while x lands and GN runs.
            for kc in range(KC):
                for _ in range(4 if kc < 3 else 6):
                    nc.tensor.matmul(wmt[:], XC[kc][:, 0:128], scr[:],
                                     start=True, stop=True)

            mes = []
            for kc in range(KC):
                me = gwork.tile([128, 2], F32, name="me", tag=f"me{kc}")
                stats = gwork.tile([128, 2, 6], F32, name="st", tag="st")
                xv = XC[kc][:].rearrange("p (s f) -> p s f", f=512)
                for shalf in range(2):
                    nc.vector.bn_stats(out=stats[:, shalf, :],
                                       in_=xv[:, shalf, :])
                mv = gwork.tile([128, 2], F32, name="mv", tag="mv")
                nc.vector.bn_aggr(out=mv, in_=stats)
                nc.vector.tensor_mul(out=me[:, 1:2], in0=mv[:, 0:1],
                                     in1=mv[:, 0:1])
                nc.vector.tensor_add(out=me[:, 1:2], in0=me[:, 1:2],
                                     in1=mv[:, 1:2])
                nc.vector.tensor_copy(out=me[:, 0:1], in_=mv[:, 0:1])
                mes.append(me)
                for _ in range(3):
                    nc.tensor.matmul(wmt[:], XC[kc][:, 256:384], scr[:],
                                     start=True, stop=True)
            # one batched chain over all 32 groups: chunk kc's group stats
            # live at COLUMNS 2kc..2kc+2 of an [8, 8] tile (all matmuls at
            # partition base 0); even columns = mean, odd = E[x^2].
            gstat = ps_gn.tile([8, 8], F32, name="gstat", tag="gstat")
            for kc in range(KC):
                nc.tensor.matmul(gstat[:, 2 * kc:2 * kc + 2], A8t,
                                 mes[kc][:], start=True, stop=True,
                                 skip_group_check=True)
            gs = gwork.tile([8, 8], F32, name="gs", tag="gs")
            nc.vector.tensor_copy(out=gs, in_=gstat)
            # chain-gated junk: a 1-element DVE write into scr creates a RAW
            # dep, so these matmuls execute HERE (not hoisted to boot),
            # covering the PE through the groupnorm scalar chain.
            nc.vector.tensor_copy(out=scr[0:1, 0:1], in_=gs[0:1, 0:1])
            junk(8)

            def ev(t):
                return t[:].rearrange("p (k two) -> p k two", two=2)[:, :, 0:1]

            def od(t):
                return t[:].rearrange("p (k two) -> p k two", two=2)[:, :, 1:2]

            def v41(t):
                return t[:].rearrange("p (k one) -> p k one", one=1)

            var = gwork.tile([8, 4], F32, name="var", tag="var")
            nc.vector.tensor_mul(out=v41(var), in0=ev(gs), in1=ev(gs))
            nc.vector.tensor_sub(out=v41(var), in0=od(gs), in1=v41(var))
            # rsqrt(var+eps): 2 Newton steps from y0=1 (GN group var ~ 1)
            w = gwork.tile([8, 4], F32, name="wv", tag="wv")
            nc.vector.tensor_scalar(out=w, in0=var, scalar1=EPS,
                                    scalar2=None, op0=OP.add)
            # Newton's y overwrites the E[x^2] columns of gs in place, so gs
            # itself becomes the [mean, rsqrt] matmul rhs (no extra copies).
            y = od(gs)
            tt = gwork.tile([8, 4], F32, name="tt", tag="tt")
            nc.vector.tensor_scalar(out=y, in0=v41(w), scalar1=-0.5,
                                    scalar2=1.5, op0=OP.mult, op1=OP.add)
            for _ in range(2):
                nc.vector.tensor_mul(out=v41(tt), in0=y, in1=y)
                nc.vector.tensor_mul(out=v41(tt), in0=v41(tt), in1=v41(w))
                nc.vector.tensor_scalar(out=tt, in0=tt, scalar1=-0.5,
                                        scalar2=1.5, op0=OP.mult,
                                        op1=OP.add)
                nc.vector.tensor_mul(out=y, in0=y, in1=v41(tt))
            cb = ps_gn.tile([128, 8], F32, name="cb", tag="cb")
            for kc in range(KC):
                nc.tensor.matmul(cb[:, 2 * kc:2 * kc + 2],
                                 E8t[:],
                                 gs[:, 2 * kc:2 * kc + 2],
                                 start=True, stop=True,
                                 skip_group_check=True)
            nc.vector.tensor_copy(out=scr[0:1, 0:1], in_=gs[0:1, 0:1])
            junk(6)
            for kc in range(KC):
                sc = gwork.tile([128, 1], F32, name=f"sc{kc}", tag=f"sc{kc}")
                sh = gwork.tile([128, 1], F32, name=f"sh{kc}", tag=f"sh{kc}")
                nc.vector.tensor_mul(out=sc, in0=cb[:, 2 * kc + 1:2 * kc + 2],
                                     in1=gnwt[:, kc:kc + 1])
                nc.vector.tensor_mul(out=sh, in0=cb[:, 2 * kc:2 * kc + 1],
                                     in1=sc)
                nc.vector.tensor_sub(out=sh, in0=gnbt[:, kc:kc + 1], in1=sh)
                if kc % 2 == 0:
                    nc.scalar.activation(
                        out=xn8[kc // 2][:, (kc % 2) * 1024:(kc % 2 + 1) * 1024],
                        in_=XC[kc][:], func=AF.Identity,
                        bias=sh[:], scale=sc[:])
                else:
                    nc.vector.tensor_scalar(
                        out=xn8[kc // 2][:, (kc % 2) * 1024:(kc % 2 + 1) * 1024],
                        in0=XC[kc][:], scalar1=sc[:], scalar2=sh[:],
                        op0=OP.mult, op1=OP.add)

        # ---------- qkv / attention / proj ----------
        with tc.tile_pool(name="ps_q", bufs=1, space="PSUM") as ps_q:
            with tc.tile_pool(name="ps_wm2", bufs=1, space="PSUM") as ps_wm2:
                wmt2 = ps_wm2.tile([128, 512], F32, name="wmt2", tag="wmt2")
                for _ in range(6):
                    nc.tensor.matmul(
                        wmt2[:],
                        xn8[0][:].rearrange("p (u n) -> p u n", u=2)[:, :, 0:128],
                        xn8[0][:].rearrange("p (u n) -> p u n", u=2)[:, :, 0:512],
                        start=True, stop=True,
                        perf_mode=DR, skip_group_check=True)
            def emit_qk_half(oc, nh, pool, tag, act=False):
                pq = pool.tile([128, 512], F32, name="pq", tag=tag)
                for kcp in range(2):
                    nc.tensor.matmul(
                        pq[:],
                        v3d(wqk8[kcp])[:, :, oc * 128:(oc + 1) * 128],
                        v3d(xn8[kcp])[:, :, nh * 512:(nh + 1) * 512],
                        start=(kcp == 0), stop=(kcp == 1),
                        perf_mode=DR, skip_group_check=True)
                    yield
                if act:
                    # head phase: bias-add on the still-idle ACT engine
                    nc.scalar.activation(
                        out=qk[oc][:, nh * 512:(nh + 1) * 512], in_=pq[:],
                        func=AF.Identity, bias=qkbt[:, oc:oc + 1], scale=1.0)
                else:
                    nc.vector.tensor_scalar(
                        out=qk[oc][:, nh * 512:(nh + 1) * 512],
                        in0=pq[:], scalar1=qkbt[:, oc:oc + 1],
                        scalar2=None, op0=OP.add)
                yield

            def emit_vt(jc, pool, tag, act=False):
                pv = pool.tile([128, 512], F32, name="pv", tag=tag)
                for kcp in range(2):
                    nc.tensor.matmul(
                        pv[:],
                        v3d(xn8[kcp])[:, :, jc * 128:(jc + 1) * 128],
                        v3d(wv8[kcp]),
                        start=(kcp == 0), stop=(kcp == 1),
                        perf_mode=DR, skip_group_check=True)
                    yield
                u = jc % 2
                vv = vt8[jc // 2][:, u * HEADS * VSTR:(u + 1) * HEADS * VSTR]
                v3 = vv.rearrange("p (h e) -> p h e", e=VSTR)
                if act:
                    nc.scalar.activation(
                        out=v3[:, :, 0:DH],
                        in_=pv[:].rearrange("p (h d) -> p h d", h=HEADS),
                        func=AF.Identity, bias=0.0, scale=1.0)
                else:
                    nc.vector.tensor_copy(
                        out=v3[:, :, 0:DH],
                        in_=pv[:].rearrange("p (h d) -> p h d", h=HEADS))
                yield

            def emit_proj_half(oc, nh, pool, tag):
                pp = pool.tile([128, 512], F32, name="pp", tag=tag)
                for prp in range(2):
                    nc.tensor.matmul(
                        pp[:],
                        v3d(pw8[prp])[:, :, oc * 128:(oc + 1) * 128],
                        v3d(att8[prp])[:, :, nh * 512:(nh + 1) * 512],
                        start=(prp == 0), stop=False,
                        perf_mode=DR, skip_group_check=True)
                    yield
                nc.tensor.matmul(
                    pp[:], eyeT[:], XC[oc][:, nh * 512:(nh + 1) * 512],
                    start=False, stop=True, skip_group_check=True)
                ot = outp.tile([128, 512], F32, name="ot", tag="ot")
                nc.vector.tensor_scalar(out=ot[:], in0=pp[:],
                                        scalar1=pbt[:, oc:oc + 1],
                                        scalar2=None, op0=OP.add)
                # mid-spine stores go via gpsimd SWDGE: a ~0.6us HWDGE
                # descriptor-gen on the scalar sequencer would delay exp
                # dispatch (sync is the semaphore hub, also avoided)
                nc.gpsimd.dma_start(
                    out=y_h[oc * 128:(oc + 1) * 128, nh * 512:(nh + 1) * 512],
                    in_=ot[:])
                yield

            def emit_filler(gen, n):
                k = 0
                for _ in range(n):
                    try:
                        next(gen)
                        k += 1
                    except StopIteration:
                        break
                return k


            # upfront: q0-nh0, k0 both halves, v0..v5
            with tc.tile_pool(name="ps_pre", bufs=4, space="PSUM") as ps_pre:
                for oc, nh, act in ((0, 0, False), (4, 0, True), (4, 1, False)):
                    emit_filler(emit_qk_half(oc, nh, ps_pre, "ppre", act=act), 99)
                for jc in range(6):
                    emit_filler(emit_vt(jc, ps_pre, "ppre", act=(jc < 2)), 99)

            def qk_stream():
                # hf-major window order: k[pr]/q[pr]-nh0 by slot 8*pr,
                # q[pr]-nh1 by slot 32+8*pr
                for oc, nh in ((5, 0), (5, 1), (1, 0)):
                    yield from emit_qk_half(oc, nh, ps_q, "pq")
                for jc in (6, 7):
                    yield from emit_vt(jc, ps_q, "pq")
                for oc, nh in ((6, 0), (6, 1), (2, 0),
                               (7, 0), (7, 1), (3, 0),
                               (0, 1), (1, 1), (2, 1), (3, 1)):
                    yield from emit_qk_half(oc, nh, ps_q, "pq")

            def proj0_stream():
                for oc in range(KC):
                    yield from emit_proj_half(oc, 0, ps_q, "pq")

            fill_qk = qk_stream()
            fill_proj = proj0_stream()

            ps_av_cm = tc.tile_pool(name="ps_av", bufs=3, space="PSUM")
            ps_av = ps_av_cm.__enter__()
            ps_s_cm = tc.tile_pool(name="ps_s", bufs=2, space="PSUM")
            ps_s = ps_s_cm.__enter__()
            if True:

                win = {}       # w -> [pav0, pav1]
                exq = {}       # global pair index -> fp8 exp pair tile

                def new_window(w):
                    win[w] = [ps_av.tile([DH + 1, 512], F32, name=f"pav{t}",
                                         tag="pav") for t in range(2)]

                def emit_av(p, ts=(0, 1)):
                    w, jcp = p // 4, p % 4
                    pr = w % 4
                    ex2 = v3d(exq[p])
                    v2 = vt8[jcp][:].rearrange("p (u f) -> p u f", u=2)
                    for t in ts:
                        h = 2 * pr + t
                        nc.tensor.matmul(
                            win[w][t][:],
                            v2[:, :, h * VSTR:h * VSTR + DH + 1],
                            ex2[:, :, t * 512:(t + 1) * 512],
                            start=(jcp == 0), stop=(jcp == 3),
                            perf_mode=DR,
                            skip_group_check=True)

                def emit_norm(w):
                    pav = win.pop(w)
                    pr, hf = w % 4, w // 4
                    base = (pr % 2) * 1024 + hf * 512
                    for t in range(2):
                        dn = recp.tile([1, 512], F32, name=f"den{t}",
                                       tag=f"den{t}")
                        nc.vector.tensor_copy(out=dn[:],
                                              in_=pav[t][DH:DH + 1, :])
                        rc = recp.tile([1, 512], F32, name=f"rec{t}",
                                       tag=f"rec{t}")
                        nc.vector.reciprocal_approx_fast(
                            out=rc[:], in_=dn[:])
                        rb = recp.tile([DH, 512], F32, name=f"rb{t}",
                                       tag=f"rb{t}")
                        nc.gpsimd.partition_broadcast(out_ap=rb[:], in_ap=rc[:])
                        nc.vector.tensor_mul(
                            out=att8[pr // 2][64 * t:64 * t + DH,
                                              base:base + 512],
                            in0=pav[t][0:DH, :],
                            in1=rb[:])

                pend = []
                for s in range(64):
                    w, jc = s // 8, s % 8
                    pr, hf = w % 4, w // 4
                    if jc == 0:
                        new_window(w)
                    qt, kt = qk[pr], qk[4 + pr]
                    pss = ps_s.tile([128, HW], F32, name="pss", tag="pss")
                    for t in range(2):
                        nc.tensor.matmul(
                            pss[:, t * 512:(t + 1) * 512],
                            kt[64 * t:64 * t + DH, jc * 128:(jc + 1) * 128],
                            qt[64 * t:64 * t + DH, hf * 512:(hf + 1) * 512],
                            start=True, stop=True)
                    for f in pend:
                        f()
                    pend = []
                    p = s // 2
                    if s % 2 == 0:
                        exq[p] = expp.tile([128, 2048], F8, name="expT",
                                           tag="expT")
                    ub = (s % 2) * 1024
                    if s == 63:
                        for t in range(2):
                            nc.scalar.activation(
                                out=exq[p][:, ub + t * 512:ub + (t + 1) * 512],
                                in_=pss[:, t * 512:(t + 1) * 512],
                                func=AF.Exp, scale=float(DH) ** -0.5,
                                bias=expb[:])
                    else:
                        nc.scalar.activation(out=exq[p][:, ub:ub + 1024],
                                             in_=pss[:],
                                             func=AF.Exp,
                                             scale=float(DH) ** -0.5,
                                             bias=expb[:])
                    if s >= 2 and s % 2 == 0:
                        pe = (s - 2) // 2
                        pend.append(lambda pe=pe: emit_av(pe, ts=(0,)))
                    elif s >= 3:
                        pe = (s - 3) // 2
                        pend.append(lambda pe=pe: (emit_av(pe, ts=(1,)),
                                                   exq.pop(pe)))
                        if pe % 4 == 3 and pe // 4 < 7:
                            pend.append(lambda ww=pe // 4: emit_norm(ww))
                    if s == 63:
                        pass
                    elif s >= 34:
                        def fmix():
                            emit_filler(fill_qk, 1)
                            emit_filler(fill_proj, 1)
                        pend.append(fmix)
                    else:
                        n = 2 if s < 6 else (1 if s % 2 == 0 else 2)
                        pend.append(lambda n=n: emit_filler(fill_qk, n))
                for f in pend:
                    f()

                # tail: last AV pair + staged final normalize: all recips
                # (straight from the PSUM denominator rows) first, then
                # broadcast+multiply per quarter, u-major so the u=0 half of
                # att[3] is released first for the split proj matmuls.
                emit_av(31)
                exq.pop(31)
                pav7 = win.pop(7)
                n_rc = []
                for u in range(2):
                    for t in range(2):
                        dn = recp.tile([1, 512], F32, name=f"den{t}", tag=f"den{t}")
                        nc.scalar.activation(
                            out=dn[:, u * 256:(u + 1) * 256],
                            in_=pav7[t][DH:DH + 1, u * 256:(u + 1) * 256],
                            func=AF.Identity, bias=0.0, scale=1.0)
                        rc = recp.tile([1, 512], F32, name=f"rec{t}", tag=f"rec{t}")
                        nc.vector.reciprocal_approx_fast(
                            out=rc[:, u * 256:(u + 1) * 256],
                            in_=dn[:, u * 256:(u + 1) * 256])
                        n_rc.append(rc)
                for i, (u, t) in enumerate(((0, 0), (0, 1), (1, 0), (1, 1))):
                    rb = recp.tile([DH, 512], F32, name=f"rb{t}", tag=f"rb{t}")
                    nc.gpsimd.partition_broadcast(
                        out_ap=rb[:, u * 256:(u + 1) * 256],
                        in_ap=n_rc[i][:, u * 256:(u + 1) * 256])
                    nc.vector.tensor_mul(
                        out=att8[1][64 * t:64 * t + DH,
                                    1536 + u * 256:1536 + (u + 1) * 256],
                        in0=pav7[t][0:DH, u * 256:(u + 1) * 256],
                        in1=rb[:, u * 256:(u + 1) * 256])
                emit_filler(fill_qk, 1000)
                emit_filler(fill_proj, 1000)

            ps_s_cm.__exit__(None, None, None)
            # proj nh=1 in the 4 banks just freed by ps_s (ps_av still open,
            # so the allocator cannot overlap the pav banks -- their release
            # waits on the final normalize reads and would stall these
            # matmuls behind it).
            with tc.tile_pool(name="ps_p2", bufs=4, space="PSUM") as ps_p2:
                pps = [ps_p2.tile([128, 512], F32, name="pp", tag="pp2")
                       for _ in range(KC)]
                for oc in range(KC):
                    nc.tensor.matmul(
                        pps[oc][:],
                        v3d(pw8[0])[:, :, oc * 128:(oc + 1) * 128],
                        v3d(att8[0])[:, :, 512:1024],
                        start=True, stop=False,
                        perf_mode=DR, skip_group_check=True)
                ots = [outp.tile([128, 512], F32, name="ot", tag="ot")
                       for _ in range(KC)]
                for u in range(2):
                    for oc in range(KC):
                        nc.tensor.matmul(
                            pps[oc][:, u * 256:(u + 1) * 256],
                            v3d(pw8[1])[:, :, oc * 128:(oc + 1) * 128],
                            v3d(att8[1])[:, :, 512 + u * 256:512 + (u + 1) * 256],
                            start=False, stop=False,
                            perf_mode=DR, skip_group_check=True)
                    for oc in range(KC):
                        nc.tensor.matmul(
                            pps[oc][:, u * 256:(u + 1) * 256],
                            eyeT[:],
                            XC[oc][:, 512 + u * 256:512 + (u + 1) * 256],
                            start=False, stop=True,
                            skip_group_check=True)
                for u in range(2):
                    for oc in range(KC):
                        if u == 0:
                            nc.vector.tensor_scalar(
                                out=ots[oc][:, 0:256],
                                in0=pps[oc][:, 0:256],
                                scalar1=pbt[:, oc:oc + 1],
                                scalar2=None, op0=OP.add)
                        else:
                            nc.scalar.activation(
                                out=ots[oc][:, 256:512],
                                in_=pps[oc][:, 256:512],
                                func=AF.Identity, bias=pbt[:, oc:oc + 1],
                                scale=1.0)
                        eng = nc.sync if (oc + u) % 2 == 0 else nc.scalar
                        eng.dma_start(
                            out=y_h[oc * 128:(oc + 1) * 128,
                                    512 + u * 256:512 + (u + 1) * 256],
                            in_=ots[oc][:, u * 256:(u + 1) * 256])
            ps_av_cm.__exit__(None, None, None)
    nc.compile()
    return nc


_NC = None


def _get_nc():
    global _NC
    if _NC is None:
        _NC = _build()
    return _NC


def _dr_pack(wT):
    """[512 c, N] -> DoubleRow fp8 layout [2*128, 2*N]:
    out[kcp*128+p, u*N+o] = wT[kcp*256+u*128+p, o]."""
    n = wT.shape[1]
    w4 = wT.reshape(2, 2, 128, n).transpose(0, 2, 1, 3).reshape(256, 2 * n)
    return np.ascontiguousarray(w4).astype(ml_dtypes.float8_e4m3)


def _run(inputs, **kwargs):
    nc = _get_nc()
    x = np.asarray(inputs["x"], dtype=np.float32)
    qkv_w = np.asarray(inputs["qkv_w"], np.float32)
    proj_w = np.asarray(inputs["proj_w"], np.float32)
    qkv_b = np.asarray(inputs["qkv_b"], np.float32)
    pb_eff = (np.asarray(inputs["proj_b"], np.float32)
              + proj_w @ qkv_b[1024:1536])
    A8_np, _ = _gn_mats()
    smalls = np.empty((128, 28), np.float32)
    smalls[:, 0:4] = np.asarray(inputs["gn_w"], np.float32).reshape(KC, 128).T
    smalls[:, 4:8] = np.asarray(inputs["gn_b"], np.float32).reshape(KC, 128).T
    smalls[:, 8:12] = pb_eff.reshape(KC, 128).T
    smalls[:, 12:20] = qkv_b[0:1024].reshape(8, 128).T
    smalls[:, 20:28] = A8_np
    shared = {
        "wqk8": _dr_pack(np.ascontiguousarray(qkv_w[0:1024].T)),
        "wv8": _dr_pack(np.ascontiguousarray(qkv_w[1024:1536].T)),
        "pw8": _dr_pack(np.ascontiguousarray(proj_w.T)),
        "smalls": smalls,
    }
    xb = x.reshape(B, C, HW).astype(ml_dtypes.bfloat16)
    in_maps = [dict(shared, xb=np.ascontiguousarray(xb[m])) for m in range(B)]
    res = run_bass_kernel_spmd(nc, in_maps, core_ids=list(range(N_CORES)), **kwargs)
    out = np.stack([res.results[m]["out"] for m in range(B)])
    return out.reshape(B, C, 32, 32).astype(np.float32), res


def kernel(**inputs):
    out, _ = _run(inputs)
    return out
